# revision 9
# baseline (speedup 1.0000x reference)
"""Chamfer distance (squared L2) Bass kernel for Trainium2, 8 NeuronCores.

Problem: xyz1 [8, 8192, 3], xyz2 [8, 8192, 3] fp32.
  out = mean_n min_m ||x_n - y_m||^2 + mean_m min_n ||x_n - y_m||^2

Sharding: batch b -> core b (8 batches, 8 cores).

Strategy (exact, windowed + verified):
  Both point sets are sorted by their x-coordinate on the host (the
  result is permutation invariant). On the device, each 128-point tile
  of sorted x computes distances only against a W=512-wide strip of
  sorted y centered on the matching rank. Since both sets are drawn
  from the same distribution, nearest neighbors are almost always
  inside the strip. Exactness is PROVEN per point on the host with the
  1-D bound d(n,m) >= (x0_n - y0_m)^2: a windowed min w is globally
  exact if w <= gap^2, where gap is the x0-distance to the nearest
  excluded candidate. The few points that fail the bound (~100 of
  16384 per batch here) are re-computed exactly on the device by a
  small patch kernel (<=256 points per direction per batch, full
  8192-wide min). If a batch ever exceeds patch capacity, a full-width
  (W=8192) variant of the same kernel - brute force, bound trivially
  satisfied - is compiled and used instead.

  Distance tiles are computed on the TensorEngine as a K=13 augmented
  matmul in fp16 with hi/lo splitting for fp32-grade accuracy:
      d[n,m] = x.z + |x|^2 + |y|^2          (z = -2y)
  (lo components scaled by 2^11, paired operand scaled by 2^-11, so all
  products stay in fp16 normal range; PSUM accumulates in fp32).
  Consecutive matmuls alternate PE row groups (base partition 0/32) so
  LDWEIGHTS overlaps the previous matmul's rhs stream. ScalarE drains
  PSUM -> SBUF fp16. VectorE keeps a running elementwise min rmin2
  (dist2) and fold-reduces each tile's rows (dist1, batched reduces).
  dist2's final min over the partition axis uses PE transposes of rmin2
  in 128x128 chunks (interleaved into the main loop once a chunk's
  coverage is complete) + batched free-axis reduce_min. Per-point
  minima return to the host, which verifies the bound, patches, sums.
"""

import numpy as np

B = 8
N = 8192
M = 8192
P = 128
NT = N // P       # 64 n-tiles
K = 13            # augmented contraction dim
SPLIT = 2048.0    # 2^11 lo-component scale
W_FAST = 512      # strip width of the fast kernel
PATCH = 256       # patch-kernel capacity (points per direction)

_COMPILED = {}


def _strip_starts(W):
    c = np.arange(NT) * P + P // 2 - W // 2
    return np.clip(c, 0, M - W).astype(np.int64)


def _build_main_nc(W):
    import concourse.mybir as mybir
    import concourse.tile as tile
    from concourse import bacc
    from concourse.masks import make_identity

    f16 = mybir.dt.float16
    f32 = mybir.dt.float32
    MIN = mybir.AluOpType.min
    X = mybir.AxisListType.X
    starts = _strip_starts(W)
    NC = M // P  # transpose chunks

    nc = bacc.Bacc("TRN2", target_bir_lowering=False, debug=False, num_devices=B)
    lhs_d = nc.dram_tensor("lhs", [K, N], f16, kind="ExternalInput").ap()
    rhs_d = nc.dram_tensor("rhs", [K, M], f16, kind="ExternalInput").ap()
    w1_d = nc.dram_tensor("w1", [P, NT], f32, kind="ExternalOutput").ap()
    w2_d = nc.dram_tensor("w2", [P, NC], f32, kind="ExternalOutput").ap()

    with tile.TileContext(nc) as tc:
        from contextlib import ExitStack

        with ExitStack() as ctx:
            cpool = ctx.enter_context(tc.tile_pool(name="const", bufs=1))
            dpool = ctx.enter_context(tc.tile_pool(name="d16", bufs=4))
            pspool = ctx.enter_context(tc.tile_pool(name="ps", bufs=4, space="PSUM"))
            ptpool = ctx.enter_context(tc.tile_pool(name="pt", bufs=2, space="PSUM"))

            lhs = cpool.tile([45, N], f16)
            rhs = cpool.tile([45, M], f16)
            nc.sync.dma_start(lhs[0:K, :], lhs_d[:])
            nc.sync.dma_start(rhs[0:K, :], rhs_d[:])
            nc.sync.dma_start(lhs[32:32 + K, :], lhs_d[:])
            nc.sync.dma_start(rhs[32:32 + K, :], rhs_d[:])

            rmin2 = cpool.tile([P, M], f16)
            nc.vector.memset(rmin2[:, 0:M // 2], float("inf"))
            nc.gpsimd.memset(rmin2[:, M // 2:M], float("inf"))
            ident = cpool.tile([P, P], f16)
            make_identity(nc, ident[:])

            rmin1 = cpool.tile([P, NT], f32)
            d2mins = cpool.tile([P, NC], f32)
            GW = min(W, 2048)  # PSUM-resident sub-group width
            half = GW // 2
            fold8 = cpool.tile([P, 8, half], f16)

            # transpose chunk c is final once no later strip covers it
            chunk_ready = np.full(NC, NT - 1, np.int64)
            for c in range(NC):
                for t in range(NT):
                    if int(starts[t]) > c * P + P - 1:
                        chunk_ready[c] = t - 1
                        break
            # schedule: at iteration nt, transpose all chunks that just
            # became final; batched reduce every 8 transposed chunks
            by_nt = [[] for _ in range(NT)]
            for c in range(NC):
                by_nt[int(chunk_ready[c])].append(c)
            tq = []  # queue of transposed-but-unreduced chunks (in order)
            n_red = [0]

            def flush_chunks(force=False):
                while len(tq) >= 8 or (force and tq):
                    grp = tq[:8]
                    del tq[:8]
                    pt = ptpool.tile([P, len(grp), P], f16, tag="pt")
                    for j, c in enumerate(grp):
                        nc.tensor.transpose(
                            pt[:, j, :], rmin2[:, c * P:(c + 1) * P], ident[:])
                    c0 = grp[0]
                    nc.vector.tensor_reduce(
                        d2mins[:, c0:c0 + len(grp)], pt[:], axis=X, op=MIN)
                    n_red[0] += 1

            mmi = 0
            for nt in range(NT):
                s = int(starts[nt])
                for g0 in range(0, W, GW):
                    ps = pspool.tile([P, GW], f32, tag="ps")
                    q0 = 0
                    while q0 < GW:
                        qw = min(512, GW - q0)
                        base = 32 * (mmi % 2)
                        mmi += 1
                        nc.tensor.matmul(
                            ps[:, q0:q0 + qw],
                            lhs[base:base + K, nt * P:(nt + 1) * P],
                            rhs[base:base + K, s + g0 + q0:s + g0 + q0 + qw],
                            start=True, stop=True)
                        q0 += qw
                    d16 = dpool.tile([P, GW], f16)
                    nc.scalar.copy(d16[:], ps[:])
                    # dist2: running min over covering strips
                    sl = rmin2[:, s + g0:s + g0 + GW]
                    nc.vector.tensor_tensor(sl, sl, d16[:], MIN)
                    # dist1: fold this tile's rows into an 8-slot staging
                    # buffer, then one batched reduce every 8 tiles
                    slot = fold8[:, nt % 8, :]
                    if g0 == 0:
                        nc.vector.tensor_tensor(
                            slot, d16[:, 0:half], d16[:, half:GW], MIN)
                    else:
                        nc.vector.tensor_tensor(
                            d16[:, 0:half], d16[:, 0:half], d16[:, half:GW], MIN)
                        nc.vector.tensor_tensor(slot, slot, d16[:, 0:half], MIN)
                if nt % 8 == 7:
                    nc.vector.tensor_reduce(
                        rmin1[:, nt - 7:nt + 1], fold8[:], axis=X, op=MIN)
                tq.extend(by_nt[nt])
                flush_chunks()
            flush_chunks(force=True)

            nc.sync.dma_start(w1_d[:], rmin1[:])
            nc.sync.dma_start(w2_d[:], d2mins[:])

    nc.compile()
    return nc


def _build_patch_nc():
    """Exact full-width mins for up to PATCH gathered points per direction.

    Direction A: points plx (stationary) vs all of mov_y (moving side).
    Direction B: points ply (stationary) vs all of mov_x.
    Output pm [128, 2 * PATCH//128 dirs-tiles] fp32, column layout
    [dirA tile0, dirA tile1, dirB tile0, dirB tile1].
    """
    import concourse.mybir as mybir
    import concourse.tile as tile
    from concourse import bacc

    f16 = mybir.dt.float16
    f32 = mybir.dt.float32
    MIN = mybir.AluOpType.min
    X = mybir.AxisListType.X
    TP = PATCH // P  # point tiles per direction

    nc = bacc.Bacc("TRN2", target_bir_lowering=False, debug=False, num_devices=B)
    plx_d = nc.dram_tensor("plx", [K, PATCH], f16, kind="ExternalInput").ap()
    ply_d = nc.dram_tensor("ply", [K, PATCH], f16, kind="ExternalInput").ap()
    movy_d = nc.dram_tensor("movy", [K, M], f16, kind="ExternalInput").ap()
    movx_d = nc.dram_tensor("movx", [K, N], f16, kind="ExternalInput").ap()
    pm_d = nc.dram_tensor("pm", [P, 2 * TP], f32, kind="ExternalOutput").ap()

    with tile.TileContext(nc) as tc:
        from contextlib import ExitStack

        with ExitStack() as ctx:
            cpool = ctx.enter_context(tc.tile_pool(name="const", bufs=1))
            dpool = ctx.enter_context(tc.tile_pool(name="pd16", bufs=4))
            pspool = ctx.enter_context(tc.tile_pool(name="pps", bufs=4, space="PSUM"))

            plx = cpool.tile([K, PATCH], f16)
            ply = cpool.tile([K, PATCH], f16)
            movy = cpool.tile([K, M], f16)
            movx = cpool.tile([K, N], f16)
            nc.sync.dma_start(plx[:], plx_d[:])
            nc.sync.dma_start(ply[:], ply_d[:])
            nc.sync.dma_start(movy[:], movy_d[:])
            nc.sync.dma_start(movx[:], movx_d[:])
            pm = cpool.tile([P, 2 * TP], f32)

            for col, (pts, mov, n_ref) in enumerate(
                    [(plx, movy, M), (ply, movx, N)]):
                for tp in range(TP):
                    acc = cpool.tile([P, 512], f16, name=f"acc{col}_{tp}")
                    lhsT = pts[:, tp * P:(tp + 1) * P]
                    for g in range(n_ref // 512):
                        ps = pspool.tile([P, 512], f32, tag="pps")
                        nc.tensor.matmul(
                            ps[:], lhsT, mov[:, g * 512:(g + 1) * 512],
                            start=True, stop=True)
                        d16 = dpool.tile([P, 512], f16)
                        nc.scalar.copy(d16[:], ps[:])
                        if g == 0:
                            nc.vector.tensor_copy(acc[:], d16[:])
                        else:
                            nc.vector.tensor_tensor(acc[:], acc[:], d16[:], MIN)
                    cc = col * TP + tp
                    nc.vector.tensor_reduce(
                        pm[:, cc:cc + 1], acc[:], axis=X, op=MIN)

            nc.sync.dma_start(pm_d[:], pm[:])

    nc.compile()
    return nc


def _side_operands(stat, mov):
    """fp16 split-precision operand rows.

    stat [Q, 3] fp32 points of the stationary side, mov [R, 3] of the
    moving side. Row pairing (STAT row k).(MOV row k), summed over k,
    yields |s|^2 + |m|^2 - 2 s.m for every (stationary, moving) pair.
    Returns STAT [13, Q], MOV [13, R].
    """
    f32 = np.float32
    f16 = np.float16

    def split(a):
        hi = a.astype(f16)
        lo_s = ((a.astype(f32) - hi.astype(f32)) * SPLIT).astype(f16)
        return hi, lo_s

    s = stat.astype(f32)
    z = (-2.0 * mov).astype(f32)
    shi, slo_s = split(s)
    zhi, zlo_s = split(z)
    shi_s = (shi.astype(f32) / SPLIT).astype(f16)
    zhi_s = (zhi.astype(f32) / SPLIT).astype(f16)
    s2 = np.square(stat.astype(np.float64)).sum(-1).astype(f32)
    m2 = np.square(mov.astype(np.float64)).sum(-1).astype(f32)
    s2hi, s2lo_s = split(s2)
    m2hi, m2lo_s = split(m2)
    ones_s = np.ones(len(s), f16)
    inv_s = np.full(len(s), 1.0 / SPLIT, f16)
    ones_m = np.ones(len(z), f16)
    inv_m = np.full(len(z), 1.0 / SPLIT, f16)

    STAT = np.stack([
        shi[:, 0], shi[:, 1], shi[:, 2],
        shi_s[:, 0], shi_s[:, 1], shi_s[:, 2],
        slo_s[:, 0], slo_s[:, 1], slo_s[:, 2],
        s2hi, s2lo_s, ones_s, inv_s])
    MOV = np.stack([
        zhi[:, 0], zhi[:, 1], zhi[:, 2],
        zlo_s[:, 0], zlo_s[:, 1], zlo_s[:, 2],
        zhi_s[:, 0], zhi_s[:, 1], zhi_s[:, 2],
        ones_m, inv_m, m2hi, m2lo_s])
    return np.ascontiguousarray(STAT), np.ascontiguousarray(MOV)


def _bound_check(w, gaps):
    """Indices whose windowed min is not provably global (fp16 slack)."""
    return np.nonzero(w.astype(np.float64) * (1 + 1e-3) + 1e-5 > gaps ** 2)[0]


def _run(xyz1, xyz2, trace=False):
    from concourse.bass_utils import run_bass_kernel_spmd

    if "main" not in _COMPILED:
        _COMPILED["main"] = _build_main_nc(W_FAST)
    if "patch" not in _COMPILED:
        _COMPILED["patch"] = _build_patch_nc()

    xyz1 = np.asarray(xyz1, dtype=np.float32)
    xyz2 = np.asarray(xyz2, dtype=np.float32)
    assert xyz1.shape == (B, N, 3) and xyz2.shape == (B, M, 3)

    starts = _strip_starts(W_FAST)
    # per-m covered n-rank range for the strip layout (same for all batches)
    cov_lo = np.full(M, M, np.int64)
    cov_hi = np.full(M, -1, np.int64)
    for t in range(NT):
        s = int(starts[t])
        cov_lo[s:s + W_FAST] = np.minimum(cov_lo[s:s + W_FAST], t * P)
        cov_hi[s:s + W_FAST] = np.maximum(cov_hi[s:s + W_FAST], (t + 1) * P - 1)

    xs = np.empty_like(xyz1)
    ys = np.empty_like(xyz2)
    stat_x = np.empty((B, K, N), np.float16)
    mov_y = np.empty((B, K, M), np.float16)
    stat_y = np.empty((B, K, M), np.float16)
    mov_x = np.empty((B, K, N), np.float16)
    for b in range(B):
        xs[b] = xyz1[b][np.argsort(xyz1[b][:, 0], kind="stable")]
        ys[b] = xyz2[b][np.argsort(xyz2[b][:, 0], kind="stable")]
        stat_x[b], mov_y[b] = _side_operands(xs[b], ys[b])
        stat_y[b], mov_x[b] = _side_operands(ys[b], xs[b])

    in_maps = [{"lhs": stat_x[b], "rhs": mov_y[b]} for b in range(B)]
    res = run_bass_kernel_spmd(_COMPILED["main"], in_maps, list(range(B)),
                               trace=trace)

    w1 = np.empty((B, N), np.float64)
    w2 = np.empty((B, M), np.float64)
    sus1 = []
    sus2 = []
    overflow = False
    for b in range(B):
        w1[b] = res.results[b]["w1"].T.reshape(-1).astype(np.float64)
        w2[b] = res.results[b]["w2"].T.reshape(-1).astype(np.float64)
        # dist1 bound: x-point vs nearest excluded sorted-y candidate
        gaps1 = np.full(N, np.inf)
        for t in range(NT):
            s = int(starts[t])
            xi = xs[b][t * P:(t + 1) * P, 0].astype(np.float64)
            lo = np.abs(xi - ys[b][s - 1, 0]) if s > 0 else np.inf
            hi = np.abs(ys[b][s + W_FAST, 0] - xi) if s + W_FAST < M else np.inf
            gaps1[t * P:(t + 1) * P] = np.minimum(lo, hi)
        # dist2 bound: y-point vs nearest excluded sorted-x candidate
        yr = ys[b][:, 0].astype(np.float64)
        lo2 = np.where(cov_lo > 0,
                       np.abs(yr - xs[b][np.maximum(cov_lo - 1, 0), 0]), np.inf)
        hi2 = np.where(cov_hi < N - 1,
                       np.abs(xs[b][np.minimum(cov_hi + 1, N - 1), 0] - yr), np.inf)
        gaps2 = np.minimum(lo2, hi2)
        i1 = _bound_check(w1[b], gaps1)
        i2 = _bound_check(w2[b], gaps2)
        if len(i1) > PATCH or len(i2) > PATCH:
            overflow = True
        sus1.append(i1)
        sus2.append(i2)

    if overflow:
        # pathological data: run the full-width (brute force) variant
        if "exact" not in _COMPILED:
            _COMPILED["exact"] = _build_main_nc(M)
        res_e = run_bass_kernel_spmd(
            _COMPILED["exact"], in_maps, list(range(B)), trace=False)
        for b in range(B):
            w1[b] = res_e.results[b]["w1"].T.reshape(-1).astype(np.float64)
            w2[b] = res_e.results[b]["w2"].T.reshape(-1).astype(np.float64)
    elif any(len(i) for i in sus1 + sus2):
        pin = []
        for b in range(B):
            i1 = np.resize(sus1[b], PATCH) if len(sus1[b]) else np.zeros(PATCH, np.int64)
            i2 = np.resize(sus2[b], PATCH) if len(sus2[b]) else np.zeros(PATCH, np.int64)
            pin.append({
                "plx": np.ascontiguousarray(stat_x[b][:, i1]),
                "ply": np.ascontiguousarray(stat_y[b][:, i2]),
                "movy": mov_y[b],
                "movx": mov_x[b],
            })
        res_p = run_bass_kernel_spmd(
            _COMPILED["patch"], pin, list(range(B)), trace=False)
        TP = PATCH // P
        for b in range(B):
            pm = res_p.results[b]["pm"]
            pa = pm[:, 0:TP].T.reshape(-1)          # dir A mins, point order
            pb = pm[:, TP:2 * TP].T.reshape(-1)     # dir B mins
            if len(sus1[b]):
                w1[b][sus1[b]] = pa[:len(sus1[b])]
            if len(sus2[b]):
                w2[b][sus2[b]] = pb[:len(sus2[b])]

    total = w1.sum() + w2.sum()
    out = np.asarray(np.float32(total / (B * N)))
    return out, res


def kernel(xyz1: np.ndarray, xyz2: np.ndarray) -> np.ndarray:
    out, _ = _run(xyz1, xyz2, trace=False)
    return out


# revision 13
# speedup vs baseline: 1.0018x; 1.0018x over previous
"""Chamfer distance (squared L2) Bass kernel for Trainium2, 8 NeuronCores.

Problem: xyz1 [8, 8192, 3], xyz2 [8, 8192, 3] fp32.
  out = mean_n min_m ||x_n - y_m||^2 + mean_m min_n ||x_n - y_m||^2

Sharding: batch b -> core b (8 batches, 8 cores).

Strategy (exact, windowed + verified):
  Both point sets are sorted by their x-coordinate on the host (the
  result is permutation invariant). On the device, each 128-point tile
  of sorted x computes distances only against a W=512-wide strip of
  sorted y centered on the matching rank. Since both sets are drawn
  from the same distribution, nearest neighbors are almost always
  inside the strip. Exactness is PROVEN per point on the host with the
  1-D bound d(n,m) >= (x0_n - y0_m)^2: a windowed min w is globally
  exact if w <= gap^2, where gap is the x0-distance to the nearest
  excluded candidate. The few points that fail the bound (~100 of
  16384 per batch here) are re-computed exactly on the device by a
  small patch kernel (<=256 points per direction per batch, full
  8192-wide min). If a batch ever exceeds patch capacity, a full-width
  (W=8192) variant of the same kernel - brute force, bound trivially
  satisfied - is compiled and used instead.

  Distance tiles are computed on the TensorEngine as a K=13 augmented
  matmul in fp16 with hi/lo splitting for fp32-grade accuracy:
      d[n,m] = x.z + |x|^2 + |y|^2          (z = -2y)
  (lo components scaled by 2^11, paired operand scaled by 2^-11, so all
  products stay in fp16 normal range; PSUM accumulates in fp32).
  Consecutive matmuls alternate PE row groups (base partition 0/32) so
  LDWEIGHTS overlaps the previous matmul's rhs stream. ScalarE drains
  PSUM -> SBUF fp16. VectorE keeps a running elementwise min rmin2
  (dist2) and fold-reduces each tile's rows (dist1, batched reduces).
  dist2's final min over the partition axis uses PE transposes of rmin2
  in 128x128 chunks (interleaved into the main loop once a chunk's
  coverage is complete) + batched free-axis reduce_min. Per-point
  minima return to the host, which verifies the bound, patches, sums.
"""

import numpy as np

B = 8
N = 8192
M = 8192
P = 128
NT = N // P       # 64 n-tiles
K = 13            # augmented contraction dim
SPLIT = 2048.0    # 2^11 lo-component scale
W_FAST = 512      # strip width of the fast kernel
PATCH = 256       # patch-kernel capacity (points per direction)

_COMPILED = {}


def _strip_starts(W):
    c = np.arange(NT) * P + P // 2 - W // 2
    return np.clip(c, 0, M - W).astype(np.int64)


def _build_main_nc(W):
    import concourse.mybir as mybir
    import concourse.tile as tile
    from concourse import bacc
    from concourse.masks import make_identity

    f16 = mybir.dt.float16
    f32 = mybir.dt.float32
    MIN = mybir.AluOpType.min
    X = mybir.AxisListType.X
    starts = _strip_starts(W)
    NC = M // P  # transpose chunks

    nc = bacc.Bacc("TRN2", target_bir_lowering=False, debug=False, num_devices=B)
    lhs_d = nc.dram_tensor("lhs", [K, N], f16, kind="ExternalInput").ap()
    rhs_d = nc.dram_tensor("rhs", [K, M], f16, kind="ExternalInput").ap()
    w1_d = nc.dram_tensor("w1", [P, NT], f32, kind="ExternalOutput").ap()
    w2_d = nc.dram_tensor("w2", [P, NC], f32, kind="ExternalOutput").ap()

    with tile.TileContext(nc) as tc:
        from contextlib import ExitStack

        with ExitStack() as ctx:
            GW = min(W, 1024)  # PSUM-resident sub-group width
            cpool = ctx.enter_context(tc.tile_pool(name="const", bufs=1))
            dpool = ctx.enter_context(tc.tile_pool(name="d16", bufs=4))
            pspool = ctx.enter_context(tc.tile_pool(
                name="ps", bufs=4 if GW <= 512 else 3, space="PSUM"))
            ptpool = ctx.enter_context(tc.tile_pool(name="pt", bufs=2, space="PSUM"))

            lhs = cpool.tile([45, N], f16)
            rhs = cpool.tile([45, M], f16)
            nc.sync.dma_start(lhs[0:K, :], lhs_d[:])
            nc.sync.dma_start(rhs[0:K, :], rhs_d[:])
            nc.sync.dma_start(lhs[32:32 + K, :], lhs_d[:])
            nc.sync.dma_start(rhs[32:32 + K, :], rhs_d[:])

            rmin2 = cpool.tile([P, M], f16)
            nc.vector.memset(rmin2[:, 0:M // 2], float("inf"))
            nc.gpsimd.memset(rmin2[:, M // 2:M], float("inf"))
            ident = cpool.tile([P, P], f16)
            make_identity(nc, ident[:])

            rmin1 = cpool.tile([P, NT], f32)
            d2mins = cpool.tile([P, NC], f32)
            half = GW // 2
            fold8 = cpool.tile([P, 8, half], f16)

            # transpose chunk c is final once no later strip covers it
            chunk_ready = np.full(NC, NT - 1, np.int64)
            for c in range(NC):
                for t in range(NT):
                    if int(starts[t]) > c * P + P - 1:
                        chunk_ready[c] = t - 1
                        break
            # schedule: at iteration nt, transpose all chunks that just
            # became final; batched reduce every 8 transposed chunks
            by_nt = [[] for _ in range(NT)]
            for c in range(NC):
                by_nt[int(chunk_ready[c])].append(c)
            tq = []  # queue of transposed-but-unreduced chunks (in order)
            n_red = [0]

            def flush_chunks(force=False):
                while len(tq) >= 8 or (force and tq):
                    grp = tq[:8]
                    del tq[:8]
                    pt = ptpool.tile([P, len(grp), P], f16, tag="pt")
                    for j, c in enumerate(grp):
                        nc.tensor.transpose(
                            pt[:, j, :], rmin2[:, c * P:(c + 1) * P], ident[:])
                    c0 = grp[0]
                    nc.vector.tensor_reduce(
                        d2mins[:, c0:c0 + len(grp)], pt[:], axis=X, op=MIN)
                    n_red[0] += 1

            mmi = 0
            for nt in range(NT):
                s = int(starts[nt])
                for g0 in range(0, W, GW):
                    ps = pspool.tile([P, GW], f32, tag="ps")
                    q0 = 0
                    while q0 < GW:
                        qw = min(512, GW - q0)
                        base = 32 * (mmi % 2)
                        mmi += 1
                        nc.tensor.matmul(
                            ps[:, q0:q0 + qw],
                            lhs[base:base + K, nt * P:(nt + 1) * P],
                            rhs[base:base + K, s + g0 + q0:s + g0 + q0 + qw],
                            start=True, stop=True)
                        q0 += qw
                    d16 = dpool.tile([P, GW], f16)
                    nc.scalar.copy(d16[:], ps[:])
                    # dist2: running min over covering strips
                    sl = rmin2[:, s + g0:s + g0 + GW]
                    nc.vector.tensor_tensor(sl, sl, d16[:], MIN)
                    # dist1: fold this tile's rows into an 8-slot staging
                    # buffer, then one batched reduce every 8 tiles
                    slot = fold8[:, nt % 8, :]
                    if g0 == 0:
                        nc.vector.tensor_tensor(
                            slot, d16[:, 0:half], d16[:, half:GW], MIN)
                    else:
                        nc.vector.tensor_tensor(
                            d16[:, 0:half], d16[:, 0:half], d16[:, half:GW], MIN)
                        nc.vector.tensor_tensor(slot, slot, d16[:, 0:half], MIN)
                if nt % 8 == 7:
                    nc.vector.tensor_reduce(
                        rmin1[:, nt - 7:nt + 1], fold8[:], axis=X, op=MIN)
                tq.extend(by_nt[nt])
                flush_chunks()
            flush_chunks(force=True)

            nc.sync.dma_start(w1_d[:], rmin1[:])
            nc.sync.dma_start(w2_d[:], d2mins[:])

    nc.compile()
    return nc


def _build_patch_nc():
    """Exact full-width mins for up to PATCH gathered points per direction.

    Direction A: points plx (stationary) vs all of mov_y (moving side).
    Direction B: points ply (stationary) vs all of mov_x.
    Output pm [128, 2 * PATCH//128 dirs-tiles] fp32, column layout
    [dirA tile0, dirA tile1, dirB tile0, dirB tile1].
    """
    import concourse.mybir as mybir
    import concourse.tile as tile
    from concourse import bacc

    f16 = mybir.dt.float16
    f32 = mybir.dt.float32
    MIN = mybir.AluOpType.min
    X = mybir.AxisListType.X
    TP = PATCH // P  # point tiles per direction

    nc = bacc.Bacc("TRN2", target_bir_lowering=False, debug=False, num_devices=B)
    plx_d = nc.dram_tensor("plx", [K, PATCH], f16, kind="ExternalInput").ap()
    ply_d = nc.dram_tensor("ply", [K, PATCH], f16, kind="ExternalInput").ap()
    movy_d = nc.dram_tensor("movy", [K, M], f16, kind="ExternalInput").ap()
    movx_d = nc.dram_tensor("movx", [K, N], f16, kind="ExternalInput").ap()
    pm_d = nc.dram_tensor("pm", [P, 2 * TP], f32, kind="ExternalOutput").ap()

    with tile.TileContext(nc) as tc:
        from contextlib import ExitStack

        with ExitStack() as ctx:
            cpool = ctx.enter_context(tc.tile_pool(name="const", bufs=1))
            dpool = ctx.enter_context(tc.tile_pool(name="pd16", bufs=4))
            pspool = ctx.enter_context(tc.tile_pool(name="pps", bufs=4, space="PSUM"))

            plx = cpool.tile([K, PATCH], f16)
            ply = cpool.tile([K, PATCH], f16)
            movy = cpool.tile([K, M], f16)
            movx = cpool.tile([K, N], f16)
            nc.sync.dma_start(plx[:], plx_d[:])
            nc.sync.dma_start(ply[:], ply_d[:])
            nc.sync.dma_start(movy[:], movy_d[:])
            nc.sync.dma_start(movx[:], movx_d[:])
            pm = cpool.tile([P, 2 * TP], f32)

            for col, (pts, mov, n_ref) in enumerate(
                    [(plx, movy, M), (ply, movx, N)]):
                for tp in range(TP):
                    acc = cpool.tile([P, 512], f16, name=f"acc{col}_{tp}")
                    lhsT = pts[:, tp * P:(tp + 1) * P]
                    for g in range(n_ref // 512):
                        ps = pspool.tile([P, 512], f32, tag="pps")
                        nc.tensor.matmul(
                            ps[:], lhsT, mov[:, g * 512:(g + 1) * 512],
                            start=True, stop=True)
                        d16 = dpool.tile([P, 512], f16)
                        nc.scalar.copy(d16[:], ps[:])
                        if g == 0:
                            nc.vector.tensor_copy(acc[:], d16[:])
                        else:
                            nc.vector.tensor_tensor(acc[:], acc[:], d16[:], MIN)
                    cc = col * TP + tp
                    nc.vector.tensor_reduce(
                        pm[:, cc:cc + 1], acc[:], axis=X, op=MIN)

            nc.sync.dma_start(pm_d[:], pm[:])

    nc.compile()
    return nc


def _side_operands(stat, mov):
    """fp16 split-precision operand rows.

    stat [Q, 3] fp32 points of the stationary side, mov [R, 3] of the
    moving side. Row pairing (STAT row k).(MOV row k), summed over k,
    yields |s|^2 + |m|^2 - 2 s.m for every (stationary, moving) pair.
    Returns STAT [13, Q], MOV [13, R].
    """
    f32 = np.float32
    f16 = np.float16

    def split(a):
        hi = a.astype(f16)
        lo_s = ((a.astype(f32) - hi.astype(f32)) * SPLIT).astype(f16)
        return hi, lo_s

    s = stat.astype(f32)
    z = (-2.0 * mov).astype(f32)
    shi, slo_s = split(s)
    zhi, zlo_s = split(z)
    shi_s = (shi.astype(f32) / SPLIT).astype(f16)
    zhi_s = (zhi.astype(f32) / SPLIT).astype(f16)
    s2 = np.square(stat.astype(np.float64)).sum(-1).astype(f32)
    m2 = np.square(mov.astype(np.float64)).sum(-1).astype(f32)
    s2hi, s2lo_s = split(s2)
    m2hi, m2lo_s = split(m2)
    ones_s = np.ones(len(s), f16)
    inv_s = np.full(len(s), 1.0 / SPLIT, f16)
    ones_m = np.ones(len(z), f16)
    inv_m = np.full(len(z), 1.0 / SPLIT, f16)

    STAT = np.stack([
        shi[:, 0], shi[:, 1], shi[:, 2],
        shi_s[:, 0], shi_s[:, 1], shi_s[:, 2],
        slo_s[:, 0], slo_s[:, 1], slo_s[:, 2],
        s2hi, s2lo_s, ones_s, inv_s])
    MOV = np.stack([
        zhi[:, 0], zhi[:, 1], zhi[:, 2],
        zlo_s[:, 0], zlo_s[:, 1], zlo_s[:, 2],
        zhi_s[:, 0], zhi_s[:, 1], zhi_s[:, 2],
        ones_m, inv_m, m2hi, m2lo_s])
    return np.ascontiguousarray(STAT), np.ascontiguousarray(MOV)


def _bound_check(w, gaps):
    """Indices whose windowed min is not provably global (fp16 slack)."""
    return np.nonzero(w.astype(np.float64) * (1 + 1e-3) + 1e-5 > gaps ** 2)[0]


def _run(xyz1, xyz2, trace=False):
    from concourse.bass_utils import run_bass_kernel_spmd

    if "main" not in _COMPILED:
        _COMPILED["main"] = _build_main_nc(W_FAST)
    if "patch" not in _COMPILED:
        _COMPILED["patch"] = _build_patch_nc()

    xyz1 = np.asarray(xyz1, dtype=np.float32)
    xyz2 = np.asarray(xyz2, dtype=np.float32)
    assert xyz1.shape == (B, N, 3) and xyz2.shape == (B, M, 3)

    starts = _strip_starts(W_FAST)
    # per-m covered n-rank range for the strip layout (same for all batches)
    cov_lo = np.full(M, M, np.int64)
    cov_hi = np.full(M, -1, np.int64)
    for t in range(NT):
        s = int(starts[t])
        cov_lo[s:s + W_FAST] = np.minimum(cov_lo[s:s + W_FAST], t * P)
        cov_hi[s:s + W_FAST] = np.maximum(cov_hi[s:s + W_FAST], (t + 1) * P - 1)

    xs = np.empty_like(xyz1)
    ys = np.empty_like(xyz2)
    stat_x = np.empty((B, K, N), np.float16)
    mov_y = np.empty((B, K, M), np.float16)
    stat_y = np.empty((B, K, M), np.float16)
    mov_x = np.empty((B, K, N), np.float16)
    for b in range(B):
        xs[b] = xyz1[b][np.argsort(xyz1[b][:, 0], kind="stable")]
        ys[b] = xyz2[b][np.argsort(xyz2[b][:, 0], kind="stable")]
        stat_x[b], mov_y[b] = _side_operands(xs[b], ys[b])
        stat_y[b], mov_x[b] = _side_operands(ys[b], xs[b])

    in_maps = [{"lhs": stat_x[b], "rhs": mov_y[b]} for b in range(B)]
    res = run_bass_kernel_spmd(_COMPILED["main"], in_maps, list(range(B)),
                               trace=trace)

    w1 = np.empty((B, N), np.float64)
    w2 = np.empty((B, M), np.float64)
    sus1 = []
    sus2 = []
    for b in range(B):
        w1[b] = res.results[b]["w1"].T.reshape(-1).astype(np.float64)
        w2[b] = res.results[b]["w2"].T.reshape(-1).astype(np.float64)
        # dist1 bound: x-point vs nearest excluded sorted-y candidate
        gaps1 = np.full(N, np.inf)
        for t in range(NT):
            s = int(starts[t])
            xi = xs[b][t * P:(t + 1) * P, 0].astype(np.float64)
            lo = np.abs(xi - ys[b][s - 1, 0]) if s > 0 else np.inf
            hi = np.abs(ys[b][s + W_FAST, 0] - xi) if s + W_FAST < M else np.inf
            gaps1[t * P:(t + 1) * P] = np.minimum(lo, hi)
        # dist2 bound: y-point vs nearest excluded sorted-x candidate
        yr = ys[b][:, 0].astype(np.float64)
        lo2 = np.where(cov_lo > 0,
                       np.abs(yr - xs[b][np.maximum(cov_lo - 1, 0), 0]), np.inf)
        hi2 = np.where(cov_hi < N - 1,
                       np.abs(xs[b][np.minimum(cov_hi + 1, N - 1), 0] - yr), np.inf)
        gaps2 = np.minimum(lo2, hi2)
        sus1.append(_bound_check(w1[b], gaps1))
        sus2.append(_bound_check(w2[b], gaps2))

    # exact patch rounds: each round fixes up to PATCH points per
    # direction per batch; loops until every suspect is re-computed
    # (one round for typical data, more only for pathological inputs)
    rounds = max([(len(i) + PATCH - 1) // PATCH for i in sus1 + sus2] + [0])
    TP = PATCH // P
    for r in range(rounds):
        pin = []
        for b in range(B):
            i1 = sus1[b][r * PATCH:(r + 1) * PATCH]
            i2 = sus2[b][r * PATCH:(r + 1) * PATCH]
            i1p = np.resize(i1, PATCH) if len(i1) else np.zeros(PATCH, np.int64)
            i2p = np.resize(i2, PATCH) if len(i2) else np.zeros(PATCH, np.int64)
            pin.append({
                "plx": np.ascontiguousarray(stat_x[b][:, i1p]),
                "ply": np.ascontiguousarray(stat_y[b][:, i2p]),
                "movy": mov_y[b],
                "movx": mov_x[b],
            })
        res_p = run_bass_kernel_spmd(
            _COMPILED["patch"], pin, list(range(B)), trace=False)
        for b in range(B):
            i1 = sus1[b][r * PATCH:(r + 1) * PATCH]
            i2 = sus2[b][r * PATCH:(r + 1) * PATCH]
            pm = res_p.results[b]["pm"]
            pa = pm[:, 0:TP].T.reshape(-1)          # dir A mins, point order
            pb = pm[:, TP:2 * TP].T.reshape(-1)     # dir B mins
            if len(i1):
                w1[b][i1] = pa[:len(i1)]
            if len(i2):
                w2[b][i2] = pb[:len(i2)]

    total = w1.sum() + w2.sum()
    out = np.asarray(np.float32(total / (B * N)))
    return out, res


def kernel(xyz1: np.ndarray, xyz2: np.ndarray) -> np.ndarray:
    out, _ = _run(xyz1, xyz2, trace=False)
    return out


# revision 16
# speedup vs baseline: 1.0439x; 1.0421x over previous
"""Chamfer distance (squared L2) Bass kernel for Trainium2, 8 NeuronCores.

Problem: xyz1 [8, 8192, 3], xyz2 [8, 8192, 3] fp32.
  out = mean_n min_m ||x_n - y_m||^2 + mean_m min_n ||x_n - y_m||^2

Sharding: batch b -> core b (8 batches, 8 cores).

Strategy (exact, windowed + verified):
  Both point sets are sorted by their x-coordinate on the host (the
  result is permutation invariant). On the device, each 128-point tile
  of sorted x computes distances only against a W=512-wide strip of
  sorted y centered on the matching rank. Since both sets are drawn
  from the same distribution, nearest neighbors are almost always
  inside the strip. Exactness is PROVEN per point on the host with the
  1-D bound d(n,m) >= (x0_n - y0_m)^2: a windowed min w is globally
  exact if w <= gap^2, where gap is the x0-distance to the nearest
  excluded candidate. The few points that fail the bound (~100 of
  16384 per batch here) are re-computed exactly on the device by a
  small patch kernel (<=256 points per direction per batch, full
  8192-wide min). If a batch ever exceeds patch capacity, a full-width
  (W=8192) variant of the same kernel - brute force, bound trivially
  satisfied - is compiled and used instead.

  Distance tiles are computed on the TensorEngine as a K=13 augmented
  matmul in fp16 with hi/lo splitting for fp32-grade accuracy:
      d[n,m] = x.z + |x|^2 + |y|^2          (z = -2y)
  (lo components scaled by 2^11, paired operand scaled by 2^-11, so all
  products stay in fp16 normal range; PSUM accumulates in fp32).
  ScalarE drains PSUM -> SBUF fp16. VectorE keeps a running min rmin2
  (dist2) and fold-reduces each tile's rows (dist1, batched reduces).
  dist2's final min over the partition axis uses PE transposes of rmin2
  in 128x128 chunks (interleaved into the main loop once a chunk's
  coverage is complete) + batched free-axis reduce_min. Per-point
  minima return to the host, which verifies the bound, patches, sums.
"""

import numpy as np

B = 8
N = 8192
M = 8192
P = 128
NT = N // P       # 64 n-tiles
K = 13            # augmented contraction dim
SPLIT = 2048.0    # 2^11 lo-component scale
W_FAST = 512      # strip width of the fast kernel
PATCH = 256       # patch-kernel capacity (points per direction)

_COMPILED = {}


def _strip_starts(W):
    c = np.arange(NT) * P + P // 2 - W // 2
    return np.clip(c, 0, M - W).astype(np.int64)


def _build_main_nc(W):
    import concourse.mybir as mybir
    import concourse.tile as tile
    from concourse import bacc
    from concourse.masks import make_identity

    f16 = mybir.dt.float16
    f32 = mybir.dt.float32
    MIN = mybir.AluOpType.min
    X = mybir.AxisListType.X
    starts = _strip_starts(W)
    NC = M // P  # transpose chunks

    nc = bacc.Bacc("TRN2", target_bir_lowering=False, debug=False, num_devices=B)
    lhs_d = nc.dram_tensor("lhs", [K, N], f16, kind="ExternalInput").ap()
    rhs_d = nc.dram_tensor("rhs", [K, M], f16, kind="ExternalInput").ap()
    w1_d = nc.dram_tensor("w1", [P, NT], f32, kind="ExternalOutput").ap()
    w2_d = nc.dram_tensor("w2", [P, NC], f32, kind="ExternalOutput").ap()

    with tile.TileContext(nc) as tc:
        from contextlib import ExitStack

        with ExitStack() as ctx:
            GW = min(W, 1024)  # PSUM-resident sub-group width
            cpool = ctx.enter_context(tc.tile_pool(name="const", bufs=1))
            dpool = ctx.enter_context(tc.tile_pool(name="d16", bufs=4))
            pspool = ctx.enter_context(tc.tile_pool(
                name="ps", bufs=4 if GW <= 512 else 3, space="PSUM"))
            ptpool = ctx.enter_context(tc.tile_pool(name="pt", bufs=2, space="PSUM"))

            lhs = cpool.tile([K, N], f16)
            rhs = cpool.tile([K, M], f16)
            nc.sync.dma_start(lhs[:], lhs_d[:])
            nc.sync.dma_start(rhs[:], rhs_d[:])

            rmin2 = cpool.tile([P, M], f16)
            nc.vector.memset(rmin2[:, 0:M // 2], float("inf"))
            nc.gpsimd.memset(rmin2[:, M // 2:M], float("inf"))
            ident = cpool.tile([P, P], f16)
            make_identity(nc, ident[:])

            rmin1 = cpool.tile([P, NT], f32)
            d2mins = cpool.tile([P, NC], f32)
            half = GW // 2
            fold8 = cpool.tile([P, 8, half], f16)

            # transpose chunk c is final once no later strip covers it
            chunk_ready = np.full(NC, NT - 1, np.int64)
            for c in range(NC):
                for t in range(NT):
                    if int(starts[t]) > c * P + P - 1:
                        chunk_ready[c] = t - 1
                        break
            # schedule: at iteration nt, transpose all chunks that just
            # became final; batched reduce every 8 transposed chunks
            by_nt = [[] for _ in range(NT)]
            for c in range(NC):
                by_nt[int(chunk_ready[c])].append(c)
            tq = []  # queue of transposed-but-unreduced chunks (in order)
            n_red = [0]

            def flush_chunks(force=False):
                while len(tq) >= 8 or (force and tq):
                    grp = tq[:8]
                    del tq[:8]
                    pt = ptpool.tile([P, len(grp), P], f16, tag="pt")
                    for j, c in enumerate(grp):
                        nc.tensor.transpose(
                            pt[:, j, :], rmin2[:, c * P:(c + 1) * P], ident[:])
                    c0 = grp[0]
                    nc.vector.tensor_reduce(
                        d2mins[:, c0:c0 + len(grp)], pt[:], axis=X, op=MIN)
                    n_red[0] += 1

            for nt in range(NT):
                s = int(starts[nt])
                for g0 in range(0, W, GW):
                    ps = pspool.tile([P, GW], f32, tag="ps")
                    for q0 in range(0, GW, 512):
                        qw = min(512, GW - q0)
                        nc.tensor.matmul(
                            ps[:, q0:q0 + qw],
                            lhs[:, nt * P:(nt + 1) * P],
                            rhs[:, s + g0 + q0:s + g0 + q0 + qw],
                            start=True, stop=True)
                    d16 = dpool.tile([P, GW], f16)
                    nc.scalar.copy(d16[:], ps[:])
                    # dist2: running min over covering strips
                    sl = rmin2[:, s + g0:s + g0 + GW]
                    nc.vector.tensor_tensor(sl, sl, d16[:], MIN)
                    # dist1: fold this tile's rows into an 8-slot staging
                    # buffer, then one batched reduce every 8 tiles
                    slot = fold8[:, nt % 8, :]
                    if g0 == 0:
                        nc.vector.tensor_tensor(
                            slot, d16[:, 0:half], d16[:, half:GW], MIN)
                    else:
                        nc.vector.tensor_tensor(
                            d16[:, 0:half], d16[:, 0:half], d16[:, half:GW], MIN)
                        nc.vector.tensor_tensor(slot, slot, d16[:, 0:half], MIN)
                if nt % 8 == 7:
                    nc.vector.tensor_reduce(
                        rmin1[:, nt - 7:nt + 1], fold8[:], axis=X, op=MIN)
                tq.extend(by_nt[nt])
                flush_chunks()
            flush_chunks(force=True)

            nc.sync.dma_start(w1_d[:], rmin1[:])
            nc.sync.dma_start(w2_d[:], d2mins[:])

    nc.compile()
    return nc


def _build_patch_nc():
    """Exact full-width mins for up to PATCH gathered points per direction.

    Direction A: points plx (stationary) vs all of mov_y (moving side).
    Direction B: points ply (stationary) vs all of mov_x.
    Output pm [128, 2 * PATCH//128 dirs-tiles] fp32, column layout
    [dirA tile0, dirA tile1, dirB tile0, dirB tile1].
    """
    import concourse.mybir as mybir
    import concourse.tile as tile
    from concourse import bacc

    f16 = mybir.dt.float16
    f32 = mybir.dt.float32
    MIN = mybir.AluOpType.min
    X = mybir.AxisListType.X
    TP = PATCH // P  # point tiles per direction

    nc = bacc.Bacc("TRN2", target_bir_lowering=False, debug=False, num_devices=B)
    plx_d = nc.dram_tensor("plx", [K, PATCH], f16, kind="ExternalInput").ap()
    ply_d = nc.dram_tensor("ply", [K, PATCH], f16, kind="ExternalInput").ap()
    movy_d = nc.dram_tensor("movy", [K, M], f16, kind="ExternalInput").ap()
    movx_d = nc.dram_tensor("movx", [K, N], f16, kind="ExternalInput").ap()
    pm_d = nc.dram_tensor("pm", [P, 2 * TP], f32, kind="ExternalOutput").ap()

    with tile.TileContext(nc) as tc:
        from contextlib import ExitStack

        with ExitStack() as ctx:
            cpool = ctx.enter_context(tc.tile_pool(name="const", bufs=1))
            dpool = ctx.enter_context(tc.tile_pool(name="pd16", bufs=4))
            pspool = ctx.enter_context(tc.tile_pool(name="pps", bufs=4, space="PSUM"))

            plx = cpool.tile([K, PATCH], f16)
            ply = cpool.tile([K, PATCH], f16)
            movy = cpool.tile([K, M], f16)
            movx = cpool.tile([K, N], f16)
            nc.sync.dma_start(plx[:], plx_d[:])
            nc.sync.dma_start(ply[:], ply_d[:])
            nc.sync.dma_start(movy[:], movy_d[:])
            nc.sync.dma_start(movx[:], movx_d[:])
            pm = cpool.tile([P, 2 * TP], f32)

            for col, (pts, mov, n_ref) in enumerate(
                    [(plx, movy, M), (ply, movx, N)]):
                for tp in range(TP):
                    acc = cpool.tile([P, 512], f16, name=f"acc{col}_{tp}")
                    lhsT = pts[:, tp * P:(tp + 1) * P]
                    for g in range(n_ref // 512):
                        ps = pspool.tile([P, 512], f32, tag="pps")
                        nc.tensor.matmul(
                            ps[:], lhsT, mov[:, g * 512:(g + 1) * 512],
                            start=True, stop=True)
                        d16 = dpool.tile([P, 512], f16)
                        nc.scalar.copy(d16[:], ps[:])
                        if g == 0:
                            nc.vector.tensor_copy(acc[:], d16[:])
                        else:
                            nc.vector.tensor_tensor(acc[:], acc[:], d16[:], MIN)
                    cc = col * TP + tp
                    nc.vector.tensor_reduce(
                        pm[:, cc:cc + 1], acc[:], axis=X, op=MIN)

            nc.sync.dma_start(pm_d[:], pm[:])

    nc.compile()
    return nc


def _side_operands(stat, mov):
    """fp16 split-precision operand rows.

    stat [Q, 3] fp32 points of the stationary side, mov [R, 3] of the
    moving side. Row pairing (STAT row k).(MOV row k), summed over k,
    yields |s|^2 + |m|^2 - 2 s.m for every (stationary, moving) pair.
    Returns STAT [13, Q], MOV [13, R].
    """
    f32 = np.float32
    f16 = np.float16

    def split(a):
        hi = a.astype(f16)
        lo_s = ((a.astype(f32) - hi.astype(f32)) * SPLIT).astype(f16)
        return hi, lo_s

    s = stat.astype(f32)
    z = (-2.0 * mov).astype(f32)
    shi, slo_s = split(s)
    zhi, zlo_s = split(z)
    shi_s = (shi.astype(f32) / SPLIT).astype(f16)
    zhi_s = (zhi.astype(f32) / SPLIT).astype(f16)
    s2 = np.square(stat.astype(np.float64)).sum(-1).astype(f32)
    m2 = np.square(mov.astype(np.float64)).sum(-1).astype(f32)
    s2hi, s2lo_s = split(s2)
    m2hi, m2lo_s = split(m2)
    ones_s = np.ones(len(s), f16)
    inv_s = np.full(len(s), 1.0 / SPLIT, f16)
    ones_m = np.ones(len(z), f16)
    inv_m = np.full(len(z), 1.0 / SPLIT, f16)

    STAT = np.stack([
        shi[:, 0], shi[:, 1], shi[:, 2],
        shi_s[:, 0], shi_s[:, 1], shi_s[:, 2],
        slo_s[:, 0], slo_s[:, 1], slo_s[:, 2],
        s2hi, s2lo_s, ones_s, inv_s])
    MOV = np.stack([
        zhi[:, 0], zhi[:, 1], zhi[:, 2],
        zlo_s[:, 0], zlo_s[:, 1], zlo_s[:, 2],
        zhi_s[:, 0], zhi_s[:, 1], zhi_s[:, 2],
        ones_m, inv_m, m2hi, m2lo_s])
    return np.ascontiguousarray(STAT), np.ascontiguousarray(MOV)


def _bound_check(w, gaps):
    """Indices whose windowed min is not provably global (fp16 slack)."""
    return np.nonzero(w.astype(np.float64) * (1 + 1e-3) + 1e-5 > gaps ** 2)[0]


def _run(xyz1, xyz2, trace=False):
    from concourse.bass_utils import run_bass_kernel_spmd

    if "main" not in _COMPILED:
        _COMPILED["main"] = _build_main_nc(W_FAST)
    if "patch" not in _COMPILED:
        _COMPILED["patch"] = _build_patch_nc()

    xyz1 = np.asarray(xyz1, dtype=np.float32)
    xyz2 = np.asarray(xyz2, dtype=np.float32)
    assert xyz1.shape == (B, N, 3) and xyz2.shape == (B, M, 3)

    starts = _strip_starts(W_FAST)
    # per-m covered n-rank range for the strip layout (same for all batches)
    cov_lo = np.full(M, M, np.int64)
    cov_hi = np.full(M, -1, np.int64)
    for t in range(NT):
        s = int(starts[t])
        cov_lo[s:s + W_FAST] = np.minimum(cov_lo[s:s + W_FAST], t * P)
        cov_hi[s:s + W_FAST] = np.maximum(cov_hi[s:s + W_FAST], (t + 1) * P - 1)

    xs = np.empty_like(xyz1)
    ys = np.empty_like(xyz2)
    stat_x = np.empty((B, K, N), np.float16)
    mov_y = np.empty((B, K, M), np.float16)
    stat_y = np.empty((B, K, M), np.float16)
    mov_x = np.empty((B, K, N), np.float16)
    for b in range(B):
        xs[b] = xyz1[b][np.argsort(xyz1[b][:, 0], kind="stable")]
        ys[b] = xyz2[b][np.argsort(xyz2[b][:, 0], kind="stable")]
        stat_x[b], mov_y[b] = _side_operands(xs[b], ys[b])
        stat_y[b], mov_x[b] = _side_operands(ys[b], xs[b])

    in_maps = [{"lhs": stat_x[b], "rhs": mov_y[b]} for b in range(B)]
    res = run_bass_kernel_spmd(_COMPILED["main"], in_maps, list(range(B)),
                               trace=trace)

    w1 = np.empty((B, N), np.float64)
    w2 = np.empty((B, M), np.float64)
    sus1 = []
    sus2 = []
    for b in range(B):
        w1[b] = res.results[b]["w1"].T.reshape(-1).astype(np.float64)
        w2[b] = res.results[b]["w2"].T.reshape(-1).astype(np.float64)
        # dist1 bound: x-point vs nearest excluded sorted-y candidate
        gaps1 = np.full(N, np.inf)
        for t in range(NT):
            s = int(starts[t])
            xi = xs[b][t * P:(t + 1) * P, 0].astype(np.float64)
            lo = np.abs(xi - ys[b][s - 1, 0]) if s > 0 else np.inf
            hi = np.abs(ys[b][s + W_FAST, 0] - xi) if s + W_FAST < M else np.inf
            gaps1[t * P:(t + 1) * P] = np.minimum(lo, hi)
        # dist2 bound: y-point vs nearest excluded sorted-x candidate
        yr = ys[b][:, 0].astype(np.float64)
        lo2 = np.where(cov_lo > 0,
                       np.abs(yr - xs[b][np.maximum(cov_lo - 1, 0), 0]), np.inf)
        hi2 = np.where(cov_hi < N - 1,
                       np.abs(xs[b][np.minimum(cov_hi + 1, N - 1), 0] - yr), np.inf)
        gaps2 = np.minimum(lo2, hi2)
        sus1.append(_bound_check(w1[b], gaps1))
        sus2.append(_bound_check(w2[b], gaps2))

    # exact patch rounds: each round fixes up to PATCH points per
    # direction per batch; loops until every suspect is re-computed
    # (one round for typical data, more only for pathological inputs)
    rounds = max([(len(i) + PATCH - 1) // PATCH for i in sus1 + sus2] + [0])
    TP = PATCH // P
    for r in range(rounds):
        pin = []
        for b in range(B):
            i1 = sus1[b][r * PATCH:(r + 1) * PATCH]
            i2 = sus2[b][r * PATCH:(r + 1) * PATCH]
            i1p = np.resize(i1, PATCH) if len(i1) else np.zeros(PATCH, np.int64)
            i2p = np.resize(i2, PATCH) if len(i2) else np.zeros(PATCH, np.int64)
            pin.append({
                "plx": np.ascontiguousarray(stat_x[b][:, i1p]),
                "ply": np.ascontiguousarray(stat_y[b][:, i2p]),
                "movy": mov_y[b],
                "movx": mov_x[b],
            })
        res_p = run_bass_kernel_spmd(
            _COMPILED["patch"], pin, list(range(B)), trace=False)
        for b in range(B):
            i1 = sus1[b][r * PATCH:(r + 1) * PATCH]
            i2 = sus2[b][r * PATCH:(r + 1) * PATCH]
            pm = res_p.results[b]["pm"]
            pa = pm[:, 0:TP].T.reshape(-1)          # dir A mins, point order
            pb = pm[:, TP:2 * TP].T.reshape(-1)     # dir B mins
            if len(i1):
                w1[b][i1] = pa[:len(i1)]
            if len(i2):
                w2[b][i2] = pb[:len(i2)]

    total = w1.sum() + w2.sum()
    out = np.asarray(np.float32(total / (B * N)))
    return out, res


def kernel(xyz1: np.ndarray, xyz2: np.ndarray) -> np.ndarray:
    out, _ = _run(xyz1, xyz2, trace=False)
    return out
